# revision 1
# baseline (speedup 1.0000x reference)
"""Trainium2 Bass kernel for nn_DenseEntangler (B=256, D=32, L=3, 6 nodes).

Math: out = relu(bias + chain of 6 tensordot contractions). Each per-sample
contraction is a (1024 x 1024) matmul applied to the reshaped state, so the
whole problem is 6 matmuls of [1024,1024]^T @ [1024, Bc*32] per core
(Bc = 32 samples/core on 8 cores, batch-sharded).

Layout scheme (verified against the reference in numpy):
  state XT[(u*32+v) partition, (b*32+f) free], K = 1024 -> 8 tiles of 128.
  steps 0..4:  OUT[(n*32+m), (b,f)] = W_i^T @ XT  with
               W_i[(u*32+v), (n*32+m)] = nodes[i][u,v,m,n]  (host pre-permute)
               transition to the next step's XT = independent aligned 32x32
               block transposes (swap partition-low m with free-low f) ->
               native DVE stream_transpose, runs off the PE critical path.
  step 5:      operands swapped (state stationary, W5 moving) so PSUM comes
               out as [(b*32+f) partition, (m*32+n) free], which is
               DRAM-contiguous per partition for the final store.
Matmuls run as float32r (full PE rate at N>=256); PSUM accumulation is fp32.
"""

import os
import sys

import numpy as np

for _p in ("/opt/trn_rl_repo", "/root/.axon_site/_ro/trn_rl_repo"):
    if _p not in sys.path and os.path.isdir(_p):
        sys.path.append(_p)

B = 256
NCORES = 8
BC = B // NCORES  # 32 samples per core
NSTEP = 6
NK = 8  # K tiles of 128 (K = 1024)
NM = 8  # output partition tiles of 128 (steps 0..4)
NHALF = 2  # halves of 16 samples -> moving free dim 512
HB = BC // NHALF  # 16

_NC_CACHE = {}


def _build_nc(mm_dtype_name):
    import concourse.tile as tile
    from concourse import bacc, mybir

    f32 = mybir.dt.float32
    mmdt = getattr(mybir.dt, mm_dtype_name)

    # Bacc (not plain Bass): its lowering runs move_matmul_waits_to_ldweights
    # + generate_event_semaphores, required to satisfy the HW 1-wait-per-
    # instruction constraint on fused LDWEIGHTS+MATMUL.
    nc = bacc.Bacc(None)
    xh = nc.declare_dram_parameter("x", [BC, 32768], f32, isOutput=False)
    wh = nc.declare_dram_parameter("w", [NSTEP, 128, 8192], f32, isOutput=False)
    bh = nc.declare_dram_parameter("bias_in", [32768], f32, isOutput=False)
    yh = nc.declare_dram_parameter("y", [BC, 32768], f32, isOutput=True)

    # x[b, (k*128+pp)*32 + f] -> [k, pp, b, f]
    x4 = xh[:, :].rearrange("b (k p f) -> k p b f", k=NK, f=32)
    # bias[(f*1024 + q)] -> [f, q]
    b2 = bh[:].rearrange("(f q) -> f q", q=1024)
    # y[b, f*1024 + q] -> [b, f, q]
    y3 = yh[:, :].rearrange("b (f q) -> b f q", q=1024)

    with tile.TileContext(nc) as tc:
        with (
            tc.tile_pool(name="wpool", bufs=16) as wpool,
            tc.tile_pool(name="xpool", bufs=32) as xpool,
            tc.tile_pool(name="bpool", bufs=1) as bpool,
            tc.tile_pool(name="tpool", bufs=4) as tpool,
            tc.tile_pool(name="stpool", bufs=4) as stpool,
            tc.tile_pool(name="opool", bufs=4) as opool,
            tc.tile_pool(name="pspool", bufs=8, space="PSUM") as pspool,
        ):
            wsb = {}

            def load_weights(step):
                # split each step's weight stream across two DGE queues:
                # even k -> gpsimd (SWDGE), odd k -> sync/scalar (HWDGE,
                # alternating by step) so the sustained weight bandwidth
                # (~148 GB/s needed) doesn't sit on a single ~154 GB/s queue.
                hw_eng = nc.sync if step % 2 == 0 else nc.scalar
                tiles = []
                for k in range(NK):
                    t = wpool.tile([128, 1024], mmdt, tag="w")
                    eng = nc.gpsimd if k % 2 == 0 else hw_eng
                    eng.dma_start(
                        out=t[:],
                        in_=wh[step, :, k * 1024 : (k + 1) * 1024].bitcast(mmdt),
                    )
                    tiles.append(t)
                wsb[step] = tiles

            # ---- head: one combined [128, 1024] x tile per k (both halves),
            # alternating the two HWDGE queues; w0 k-tiles interleaved with
            # gpsimd taking the even ones. PE consumes k in arrival order.
            x0 = [None] * NK
            wsb[0] = []
            for k in range(NK):
                tx = xpool.tile([128, BC * 32], mmdt, tag="x0", name=f"x0_{k}", bufs=8)
                # strided loads run at only ~40 GB/s per queue; alternate the
                # two HWDGE queues per (k, half) so each half-stream gets 2x.
                qa, qb = (nc.sync, nc.scalar) if k % 2 == 0 else (nc.scalar, nc.sync)
                qa.dma_start(
                    out=tx[:, 0 : HB * 32].rearrange("p (b f) -> p b f", f=32),
                    in_=x4[k, :, 0:HB, :].bitcast(mmdt),
                )
                qb.dma_start(
                    out=tx[:, HB * 32 : BC * 32].rearrange("p (b f) -> p b f", f=32),
                    in_=x4[k, :, HB:BC, :].bitcast(mmdt),
                )
                x0[k] = tx
                t = wpool.tile([128, 1024], mmdt, tag="w", name=f"w0_{k}")
                nc.gpsimd.dma_start(
                    out=t[:], in_=wh[0, :, k * 1024 : (k + 1) * 1024].bitcast(mmdt)
                )
                wsb[0].append(t)

            load_weights(1)

            def finish_tile(ps, h, mt, xt_next):
                """PSUM -> (transpose, round-to-mmdt) -> next-step state tile."""
                if mmdt is f32:
                    t = xpool.tile([128, 512], f32, tag="xt")
                    nc.vector.transpose(t[:], ps[:])
                else:
                    st = stpool.tile([128, 512], f32, tag="st")
                    nc.vector.transpose(st[:], ps[:])
                    t = xpool.tile([128, 512], mmdt, tag="xt")
                    nc.scalar.copy(t[:], st[:])
                xt_next[h][mt] = t

            # ---- step 0: k-outer so PE consumes k-tiles in DMA arrival order
            xt_next = [[None] * NK for _ in range(NHALF)]
            for h in range(NHALF):
                pss = [
                    pspool.tile([128, 512], f32, tag="ps", name=f"ps0_{h}_{i}")
                    for i in range(NM)
                ]
                for k in range(NK):
                    for mt in range(NM):
                        nc.tensor.matmul(
                            pss[mt][:],
                            wsb[0][k][:, mt * 128 : (mt + 1) * 128],
                            x0[k][:, h * 512 : (h + 1) * 512],
                            start=(k == 0),
                            stop=(k == NK - 1),
                        )
                for mt in range(NM):
                    finish_tile(pss[mt], h, mt, xt_next)
            load_weights(2)
            xt = xt_next

            # ---- steps 1..4: mt-outer (staggers transposes across the step)
            for step in range(1, 5):
                xt_next = [[None] * NK for _ in range(NHALF)]
                for h in range(NHALF):
                    for mt in range(NM):
                        ps = pspool.tile([128, 512], f32, tag="ps")
                        for k in range(NK):
                            nc.tensor.matmul(
                                ps[:],
                                wsb[step][k][:, mt * 128 : (mt + 1) * 128],
                                xt[h][k][:],
                                start=(k == 0),
                                stop=(k == NK - 1),
                            )
                        finish_tile(ps, h, mt, xt_next)
                if step + 2 < NSTEP:
                    load_weights(step + 2)
                xt = xt_next

            # ---- step 5: state stationary, W moving; fused bias+relu+store ----
            from concourse.mybir import ActivationFunctionType

            # bias tile: [128, 1024], row p holds bias[(p%32)*1024 : ...];
            # loaded late, right before its only consumer.
            bias_sb = bpool.tile([128, 1024], f32, tag="bias")
            for r in range(4):
                nc.sync.dma_start(out=bias_sb[32 * r : 32 * (r + 1), :], in_=b2[:, :])

            for h in range(NHALF):
                for mc in range(4):  # output partition chunks of 128 (= 4 b values)
                    for nh in range(2):  # N halves of 512
                        ps = pspool.tile([128, 512], f32, tag="ps")
                        for k in range(NK):
                            nc.tensor.matmul(
                                ps[:],
                                xt[h][k][:, mc * 128 : (mc + 1) * 128],
                                wsb[5][k][:, nh * 512 : (nh + 1) * 512],
                                start=(k == 0),
                                stop=(k == NK - 1),
                            )
                        tmp = tpool.tile([128, 512], f32, tag="tmp")
                        nc.vector.tensor_add(
                            tmp[:], ps[:], bias_sb[:, nh * 512 : (nh + 1) * 512]
                        )
                        o = opool.tile([128, 512], f32, tag="o")
                        nc.scalar.activation(o[:], tmp[:], ActivationFunctionType.Relu)
                        b0 = h * HB + mc * 4
                        nc.sync.dma_start(
                            out=y3[b0 : b0 + 4, :, nh * 512 : (nh + 1) * 512],
                            in_=o[:],
                        )
    # Run the Bacc lowering passes (register allocation, wait splitting, ...)
    # — the PJRT execute path serializes nc.m as-is.
    nc.finalize()
    return nc


def _get_nc(mm_dtype_name):
    if mm_dtype_name not in _NC_CACHE:
        _NC_CACHE[mm_dtype_name] = _build_nc(mm_dtype_name)
    return _NC_CACHE[mm_dtype_name]


def _prep_weights(nodes):
    # W[i] layout [p=(u*32+v)%... rows 128 per k-tile packed as [128, 8*1024]]:
    # free index = k*1024 + col.  steps 0..4: col = n*32+m ; step 5: col = m*32+n.
    nodes = np.ascontiguousarray(nodes, dtype=np.float32)
    W = np.empty((NSTEP, 128, 8192), np.float32)
    for i in range(NSTEP):
        if i < 5:
            wm = nodes[i].reshape(1024, 32, 32).transpose(0, 2, 1).reshape(1024, 1024)
        else:
            wm = nodes[i].reshape(1024, 1024)
        # [k*128+p, col] -> [p, k*1024+col]
        W[i] = wm.reshape(NK, 128, 1024).transpose(1, 0, 2).reshape(128, 8192)
    return W


def run(inputs, nodes, bias, mm_dtype="float32r", trace=False):
    from concourse.bass_utils import run_bass_kernel_spmd

    nc = _get_nc(mm_dtype)
    x = np.ascontiguousarray(inputs, dtype=np.float32)
    bias = np.ascontiguousarray(bias, dtype=np.float32)
    W = _prep_weights(nodes)
    in_maps = [
        {"x": x[c * BC : (c + 1) * BC], "w": W, "bias_in": bias}
        for c in range(NCORES)
    ]
    res = run_bass_kernel_spmd(nc, in_maps, list(range(NCORES)), trace=trace)
    out = np.concatenate([res.results[c]["y"] for c in range(NCORES)], axis=0)
    return out, res


def kernel(inputs, nodes, bias):
    out, _ = run(inputs, nodes, bias)
    return out



# revision 4
# speedup vs baseline: 1.1863x; 1.1863x over previous
"""Trainium2 Bass kernel for nn_DenseEntangler (B=256, D=32, L=3, 6 nodes).

Math: out = relu(bias + chain of 6 tensordot contractions). Each per-sample
contraction is a (1024 x 1024) matmul applied to the reshaped state, so the
whole problem is 6 matmuls of [1024,1024]^T @ [1024, Bc*32] per core
(Bc = 32 samples/core on 8 cores, batch-sharded).

Layout scheme (verified against the reference in numpy):
  state XT[(u*32+v) partition, (b*32+f) free], K = 1024 -> 8 tiles of 128.
  steps 0..4:  OUT[(n*32+m), (b,f)] = W_i^T @ XT  with
               W_i[(u*32+v), (n*32+m)] = nodes[i][u,v,m,n]  (host pre-permute)
               transition to the next step's XT = independent aligned 32x32
               block transposes (swap partition-low m with free-low f) ->
               native DVE stream_transpose, runs off the PE critical path.
  step 5:      operands swapped (state stationary, W5 moving) so PSUM comes
               out as [(b*32+f) partition, (m*32+n) free], which is
               DRAM-contiguous per partition for the final store.

Perf notes (v2): matmuls run in bf16 (1 cycle/row on the PE, identical to
float32r at N>=256, but FWL hides the weight loads and DMA bytes halve);
PSUM accumulation stays fp32. x is pre-permuted ON THE HOST into the exact
SBUF tile layout [k, p, b*f] so the head DMA is fully contiguous -- the
fp32r baseline lost ~50us to ~40GB/s strided gathers at the head. All six
weight sets are resident in SBUF (bf16 halves their footprint), loaded
up-front across the two HWDGE rings + gpsimd SWDGE.
"""

import os
import sys

import numpy as np

for _p in ("/opt/trn_rl_repo", "/root/.axon_site/_ro/trn_rl_repo"):
    if _p not in sys.path and os.path.isdir(_p):
        sys.path.append(_p)

B = 256
NCORES = 8
BC = B // NCORES  # 32 samples per core
NSTEP = 6
NK = 8  # K tiles of 128 (K = 1024)
NM = 8  # output partition tiles of 128 (steps 0..4)
NHALF = 2  # halves of 16 samples -> moving free dim 512
HB = BC // NHALF  # 16

_NC_CACHE = {}


def _np_dtype(mm_dtype_name):
    if mm_dtype_name in ("float32", "float32r"):
        return np.float32
    from ml_dtypes import bfloat16

    assert mm_dtype_name == "bfloat16", mm_dtype_name
    return bfloat16


def _build_nc(mm_dtype_name):
    import concourse.tile as tile
    from concourse import bacc, mybir

    f32 = mybir.dt.float32
    mmdt = getattr(mybir.dt, mm_dtype_name)
    # DRAM declaration dtype: f32 for the 4-byte paths (f32/f32r share bits),
    # bf16 natively otherwise. `cast` bitcasts an AP only when needed.
    ddt = f32 if mm_dtype_name in ("float32", "float32r") else mmdt
    cast = (lambda ap: ap.bitcast(mmdt)) if mmdt != ddt else (lambda ap: ap)

    # Bacc (not plain Bass): its lowering runs move_matmul_waits_to_ldweights
    # + generate_event_semaphores, required to satisfy the HW 1-wait-per-
    # instruction constraint on fused LDWEIGHTS+MATMUL.
    nc = bacc.Bacc(None)
    # x arrives pre-permuted from the host: x3[k, p, b*32+f] = x[b, (k*128+p)*32+f]
    xh = nc.declare_dram_parameter("x", [NK, 128, BC * 32], ddt, isOutput=False)
    wh = nc.declare_dram_parameter("w", [NSTEP, 128, NK * 1024], ddt, isOutput=False)
    bh = nc.declare_dram_parameter("bias_in", [32768], f32, isOutput=False)
    yh = nc.declare_dram_parameter("y", [BC, 32768], f32, isOutput=True)

    # bias[(f*1024 + q)] -> [f, q]
    b2 = bh[:].rearrange("(f q) -> f q", q=1024)
    # y[b, f*1024 + q] -> [b, f, q]
    y3 = yh[:, :].rearrange("b (f q) -> b f q", q=1024)

    with tile.TileContext(nc) as tc:
        with (
            tc.tile_pool(name="wpool", bufs=8) as wpool,
            tc.tile_pool(name="xpool", bufs=32) as xpool,
            tc.tile_pool(name="bpool", bufs=1) as bpool,
            tc.tile_pool(name="tpool", bufs=4) as tpool,
            tc.tile_pool(name="stpool", bufs=4) as stpool,
            tc.tile_pool(name="opool", bufs=4) as opool,
            tc.tile_pool(name="pspool", bufs=8, space="PSUM") as pspool,
        ):
            wsb = {s: [None] * NK for s in range(NSTEP)}

            # ---- head: interleave x[k] and w0[k] on the two HWDGE rings so
            # the PE's (x[k], w0[k]) pairs arrive in consumption order. Both
            # transfers are fully contiguous in DRAM (256KB each in bf16).
            x0 = [None] * NK
            for k in range(NK):
                qa, qb = (nc.sync, nc.scalar) if k % 2 == 0 else (nc.scalar, nc.sync)
                tx = xpool.tile([128, BC * 32], mmdt, tag="x0", name=f"x0_{k}", bufs=8)
                qa.dma_start(out=tx[:], in_=cast(xh[k, :, :]))
                x0[k] = tx
                t = wpool.tile([128, 1024], mmdt, tag="w", name=f"w0_{k}")
                qb.dma_start(out=t[:], in_=cast(wh[0, :, k * 1024 : (k + 1) * 1024]))
                wsb[0][k] = t

            # ---- all later weight sets, issued up-front; they stream in
            # behind the head on whichever ring has slack and are all
            # resident well before their step begins. W1 goes via gpsimd
            # (SWDGE) in large chunks so it does not queue behind the head.
            # per-tag bufs: tag sizes to bufs*max(size), so each chunk width
            # gets its own tag ("w4": 4 bufs of [128,4096]; "w8": 3 bufs of
            # [128,8192]); step-0's [128,1024] tiles live in the default "w".
            _chunk_bufs = {2: ("w4", 4), 1: ("w8", 3)}

            def load_weights(step, eng, nchunk):
                per = NK // nchunk
                wtag, wbufs = _chunk_bufs[nchunk]
                for c in range(nchunk):
                    t = wpool.tile(
                        [128, per * 1024],
                        mmdt,
                        tag=wtag,
                        bufs=wbufs,
                        name=f"w{step}_{c}",
                    )
                    eng.dma_start(
                        out=t[:],
                        in_=cast(
                            wh[step, :, c * per * 1024 : (c + 1) * per * 1024]
                        ),
                    )
                    for k in range(per):
                        wsb[step][c * per + k] = t[:, k * 1024 : (k + 1) * 1024]

            load_weights(1, nc.gpsimd, 2)
            load_weights(2, nc.gpsimd, 2)
            load_weights(3, nc.sync, 1)
            load_weights(4, nc.scalar, 1)
            load_weights(5, nc.sync, 1)

            def finish_tile(ps, h, mt, xt_next):
                """PSUM -> (transpose, round-to-mmdt) -> next-step state tile."""
                if mmdt is f32:
                    t = xpool.tile([128, 512], f32, tag="xt")
                    nc.vector.transpose(t[:], ps[:])
                else:
                    st = stpool.tile([128, 512], f32, tag="st")
                    nc.vector.transpose(st[:], ps[:])
                    t = xpool.tile([128, 512], mmdt, tag="xt")
                    nc.scalar.copy(t[:], st[:])
                xt_next[h][mt] = t

            # ---- step 0: k-outer so PE consumes k-tiles in DMA arrival order
            xt_next = [[None] * NK for _ in range(NHALF)]
            for h in range(NHALF):
                pss = [
                    pspool.tile([128, 512], f32, tag="ps", name=f"ps0_{h}_{i}")
                    for i in range(NM)
                ]
                for k in range(NK):
                    for mt in range(NM):
                        nc.tensor.matmul(
                            pss[mt][:],
                            wsb[0][k][:, mt * 128 : (mt + 1) * 128],
                            x0[k][:, h * 512 : (h + 1) * 512],
                            start=(k == 0),
                            stop=(k == NK - 1),
                        )
                for mt in range(NM):
                    finish_tile(pss[mt], h, mt, xt_next)
            xt = xt_next

            # ---- steps 1..4: mt-outer (staggers transposes across the step)
            for step in range(1, 5):
                xt_next = [[None] * NK for _ in range(NHALF)]
                for h in range(NHALF):
                    for mt in range(NM):
                        ps = pspool.tile([128, 512], f32, tag="ps")
                        for k in range(NK):
                            nc.tensor.matmul(
                                ps[:],
                                wsb[step][k][:, mt * 128 : (mt + 1) * 128],
                                xt[h][k][:],
                                start=(k == 0),
                                stop=(k == NK - 1),
                            )
                        finish_tile(ps, h, mt, xt_next)
                xt = xt_next

            # ---- step 5: state stationary, W moving; fused bias+relu+store ----
            from concourse.mybir import ActivationFunctionType

            # bias tile: [128, 1024], row p holds bias[(p%32)*1024 : ...];
            # loaded late, right before its only consumer.
            bias_sb = bpool.tile([128, 1024], f32, tag="bias")
            for r in range(4):
                nc.sync.dma_start(out=bias_sb[32 * r : 32 * (r + 1), :], in_=b2[:, :])

            for h in range(NHALF):
                for mc in range(4):  # output partition chunks of 128 (= 4 b values)
                    for nh in range(2):  # N halves of 512
                        ps = pspool.tile([128, 512], f32, tag="ps")
                        for k in range(NK):
                            nc.tensor.matmul(
                                ps[:],
                                xt[h][k][:, mc * 128 : (mc + 1) * 128],
                                wsb[5][k][:, nh * 512 : (nh + 1) * 512],
                                start=(k == 0),
                                stop=(k == NK - 1),
                            )
                        tmp = tpool.tile([128, 512], f32, tag="tmp")
                        nc.vector.tensor_add(
                            tmp[:], ps[:], bias_sb[:, nh * 512 : (nh + 1) * 512]
                        )
                        o = opool.tile([128, 512], f32, tag="o")
                        nc.scalar.activation(o[:], tmp[:], ActivationFunctionType.Relu)
                        b0 = h * HB + mc * 4
                        eng = nc.sync if (mc + nh) % 2 == 0 else nc.scalar
                        eng.dma_start(
                            out=y3[b0 : b0 + 4, :, nh * 512 : (nh + 1) * 512],
                            in_=o[:],
                        )
    # Run the Bacc lowering passes (register allocation, wait splitting, ...)
    # — the PJRT execute path serializes nc.m as-is.
    nc.finalize()
    return nc


def _get_nc(mm_dtype_name):
    if mm_dtype_name not in _NC_CACHE:
        _NC_CACHE[mm_dtype_name] = _build_nc(mm_dtype_name)
    return _NC_CACHE[mm_dtype_name]


def _prep_weights(nodes, npdt):
    # W[i] layout [p=(u*32+v)%... rows 128 per k-tile packed as [128, 8*1024]]:
    # free index = k*1024 + col.  steps 0..4: col = n*32+m ; step 5: col = m*32+n.
    nodes = np.ascontiguousarray(nodes, dtype=np.float32)
    W = np.empty((NSTEP, 128, 8192), np.float32)
    for i in range(NSTEP):
        if i < 5:
            wm = nodes[i].reshape(1024, 32, 32).transpose(0, 2, 1).reshape(1024, 1024)
        else:
            wm = nodes[i].reshape(1024, 1024)
        # [k*128+p, col] -> [p, k*1024+col]
        W[i] = wm.reshape(NK, 128, 1024).transpose(1, 0, 2).reshape(128, 8192)
    return np.ascontiguousarray(W.astype(npdt))


def _prep_x(xc, npdt):
    # [BC, 32768] -> [k, p, b*32+f] with value x[b, (k*128+p)*32+f]
    xp = xc.reshape(BC, NK, 128, 32).transpose(1, 2, 0, 3).reshape(NK, 128, BC * 32)
    return np.ascontiguousarray(xp.astype(npdt))


def run(inputs, nodes, bias, mm_dtype="bfloat16", trace=False):
    from concourse.bass_utils import run_bass_kernel_spmd

    nc = _get_nc(mm_dtype)
    npdt = _np_dtype(mm_dtype)
    x = np.ascontiguousarray(inputs, dtype=np.float32)
    bias = np.ascontiguousarray(bias, dtype=np.float32)
    W = _prep_weights(nodes, npdt)
    in_maps = [
        {
            "x": _prep_x(x[c * BC : (c + 1) * BC], npdt),
            "w": W,
            "bias_in": bias,
        }
        for c in range(NCORES)
    ]
    res = run_bass_kernel_spmd(nc, in_maps, list(range(NCORES)), trace=trace)
    out = np.concatenate([res.results[c]["y"] for c in range(NCORES)], axis=0)
    return out, res


def kernel(inputs, nodes, bias):
    out, _ = run(inputs, nodes, bias)
    return out


# revision 5
# speedup vs baseline: 1.2618x; 1.0636x over previous
"""Trainium2 Bass kernel for nn_DenseEntangler (B=256, D=32, L=3, 6 nodes).

Math: out = relu(bias + chain of 6 tensordot contractions). Each per-sample
contraction is a (1024 x 1024) matmul applied to the reshaped state, so the
whole problem is 6 matmuls of [1024,1024]^T @ [1024, Bc*32] per core
(Bc = 32 samples/core on 8 cores, batch-sharded).

Layout scheme (verified against the reference in numpy):
  state XT[(u*32+v) partition, (b*32+f) free], K = 1024 -> 8 tiles of 128.
  steps 0..4:  OUT[(n*32+m), (b,f)] = W_i^T @ XT  with
               W_i[(u*32+v), (n*32+m)] = nodes[i][u,v,m,n]  (host pre-permute)
               transition to the next step's XT = independent aligned 32x32
               block transposes (swap partition-low m with free-low f) ->
               native DVE stream_transpose, runs off the PE critical path.
  step 5:      operands swapped (state stationary, W5 moving) so PSUM comes
               out as [(b*32+f) partition, (m*32+n) free], which is
               DRAM-contiguous per partition for the final store.

Perf notes (v2): matmuls run in bf16 (1 cycle/row on the PE, identical to
float32r at N>=256, but FWL hides the weight loads and DMA bytes halve);
PSUM accumulation stays fp32. x is pre-permuted ON THE HOST into the exact
SBUF tile layout [k, p, b*f] so the head DMA is fully contiguous -- the
fp32r baseline lost ~50us to ~40GB/s strided gathers at the head. All six
weight sets are resident in SBUF (bf16 halves their footprint), loaded
up-front across the two HWDGE rings + gpsimd SWDGE.
"""

import os
import sys

import numpy as np

for _p in ("/opt/trn_rl_repo", "/root/.axon_site/_ro/trn_rl_repo"):
    if _p not in sys.path and os.path.isdir(_p):
        sys.path.append(_p)

B = 256
NCORES = 8
BC = B // NCORES  # 32 samples per core
NSTEP = 6
NK = 8  # K tiles of 128 (K = 1024)
NM = 8  # output partition tiles of 128 (steps 0..4)
NHALF = 2  # halves of 16 samples -> moving free dim 512
HB = BC // NHALF  # 16

_NC_CACHE = {}


def _np_dtype(mm_dtype_name):
    if mm_dtype_name in ("float32", "float32r"):
        return np.float32
    from ml_dtypes import bfloat16

    assert mm_dtype_name == "bfloat16", mm_dtype_name
    return bfloat16


def _build_nc(mm_dtype_name):
    import concourse.tile as tile
    from concourse import bacc, mybir

    f32 = mybir.dt.float32
    mmdt = getattr(mybir.dt, mm_dtype_name)
    # DRAM declaration dtype: f32 for the 4-byte paths (f32/f32r share bits),
    # bf16 natively otherwise. `cast` bitcasts an AP only when needed.
    ddt = f32 if mm_dtype_name in ("float32", "float32r") else mmdt
    cast = (lambda ap: ap.bitcast(mmdt)) if mmdt != ddt else (lambda ap: ap)

    # Bacc (not plain Bass): its lowering runs move_matmul_waits_to_ldweights
    # + generate_event_semaphores, required to satisfy the HW 1-wait-per-
    # instruction constraint on fused LDWEIGHTS+MATMUL.
    nc = bacc.Bacc(None)
    # x arrives pre-permuted from the host: x3[k, p, b*32+f] = x[b, (k*128+p)*32+f]
    xh = nc.declare_dram_parameter("x", [NK, 128, BC * 32], ddt, isOutput=False)
    wh = nc.declare_dram_parameter("w", [NSTEP, 128, NK * 1024], ddt, isOutput=False)
    bh = nc.declare_dram_parameter("bias_in", [32768], f32, isOutput=False)
    yh = nc.declare_dram_parameter("y", [BC, 32768], f32, isOutput=True)

    # bias[(f*1024 + q)] -> [f, q]
    b2 = bh[:].rearrange("(f q) -> f q", q=1024)
    # y[b, f*1024 + q] -> [b, f, q]
    y3 = yh[:, :].rearrange("b (f q) -> b f q", q=1024)

    with tile.TileContext(nc) as tc:
        with (
            tc.tile_pool(name="wpool", bufs=8) as wpool,
            tc.tile_pool(name="xpool", bufs=32) as xpool,
            tc.tile_pool(name="bpool", bufs=1) as bpool,
            tc.tile_pool(name="tpool", bufs=4) as tpool,
            tc.tile_pool(name="stpool", bufs=4) as stpool,
            tc.tile_pool(name="opool", bufs=4) as opool,
            tc.tile_pool(name="pspool", bufs=8, space="PSUM") as pspool,
        ):
            wsb = {s: [None] * NK for s in range(NSTEP)}

            # ---- head: interleave x[k] and w0[k] on the two HWDGE rings so
            # the PE's (x[k], w0[k]) pairs arrive in consumption order. Both
            # transfers are fully contiguous in DRAM (256KB each in bf16).
            x0 = [None] * NK
            for k in range(NK):
                qa, qb = (nc.sync, nc.scalar) if k % 2 == 0 else (nc.scalar, nc.sync)
                tx = xpool.tile([128, BC * 32], mmdt, tag="x0", name=f"x0_{k}", bufs=8)
                qa.dma_start(out=tx[:], in_=cast(xh[k, :, :]))
                x0[k] = tx
                t = wpool.tile([128, 1024], mmdt, tag="w", name=f"w0_{k}")
                qb.dma_start(out=t[:], in_=cast(wh[0, :, k * 1024 : (k + 1) * 1024]))
                wsb[0][k] = t

            # ---- all later weight sets, issued up-front but strictly BEHIND
            # the head tiles on the two FIFO HWDGE rings, ordered by the time
            # each step needs them. The head (x+w0, 4MB) gets the full
            # ~358GB/s HBM budget and drains in ~11us < step-0 compute; each
            # W_i (2MB per ring slot) lands long before its step starts.
            # gpsimd stays idle so SWDGE doesn't steal head bandwidth.
            # (tag note: a tag sizes to bufs*max(size), so the [128,8192]
            # chunks get their own tag "w8" with one buf per step 1..5.)
            def load_weights(step, eng):
                t = wpool.tile(
                    [128, NK * 1024], mmdt, tag="w8", bufs=5, name=f"w{step}"
                )
                eng.dma_start(out=t[:], in_=cast(wh[step, :, :]))
                for k in range(NK):
                    wsb[step][k] = t[:, k * 1024 : (k + 1) * 1024]

            load_weights(1, nc.sync)
            load_weights(2, nc.scalar)
            load_weights(3, nc.sync)
            load_weights(4, nc.scalar)
            load_weights(5, nc.sync)

            def finish_tile(ps, h, mt, xt_next):
                """PSUM -> (transpose, round-to-mmdt) -> next-step state tile."""
                if mmdt is f32:
                    t = xpool.tile([128, 512], f32, tag="xt")
                    nc.vector.transpose(t[:], ps[:])
                else:
                    st = stpool.tile([128, 512], f32, tag="st")
                    nc.vector.transpose(st[:], ps[:])
                    t = xpool.tile([128, 512], mmdt, tag="xt")
                    nc.scalar.copy(t[:], st[:])
                xt_next[h][mt] = t

            # ---- step 0: k-outer so PE consumes k-tiles in DMA arrival order
            xt_next = [[None] * NK for _ in range(NHALF)]
            for h in range(NHALF):
                pss = [
                    pspool.tile([128, 512], f32, tag="ps", name=f"ps0_{h}_{i}")
                    for i in range(NM)
                ]
                for k in range(NK):
                    for mt in range(NM):
                        nc.tensor.matmul(
                            pss[mt][:],
                            wsb[0][k][:, mt * 128 : (mt + 1) * 128],
                            x0[k][:, h * 512 : (h + 1) * 512],
                            start=(k == 0),
                            stop=(k == NK - 1),
                        )
                for mt in range(NM):
                    finish_tile(pss[mt], h, mt, xt_next)
            xt = xt_next

            # ---- steps 1..4: mt-outer (staggers transposes across the step)
            for step in range(1, 5):
                xt_next = [[None] * NK for _ in range(NHALF)]
                for h in range(NHALF):
                    for mt in range(NM):
                        ps = pspool.tile([128, 512], f32, tag="ps")
                        for k in range(NK):
                            nc.tensor.matmul(
                                ps[:],
                                wsb[step][k][:, mt * 128 : (mt + 1) * 128],
                                xt[h][k][:],
                                start=(k == 0),
                                stop=(k == NK - 1),
                            )
                        finish_tile(ps, h, mt, xt_next)
                xt = xt_next

            # ---- step 5: state stationary, W moving; fused bias+relu+store ----
            from concourse.mybir import ActivationFunctionType

            # bias tile: [128, 1024], row p holds bias[(p%32)*1024 : ...];
            # loaded late, right before its only consumer.
            bias_sb = bpool.tile([128, 1024], f32, tag="bias")
            for r in range(4):
                nc.sync.dma_start(out=bias_sb[32 * r : 32 * (r + 1), :], in_=b2[:, :])

            for h in range(NHALF):
                for mc in range(4):  # output partition chunks of 128 (= 4 b values)
                    for nh in range(2):  # N halves of 512
                        ps = pspool.tile([128, 512], f32, tag="ps")
                        for k in range(NK):
                            nc.tensor.matmul(
                                ps[:],
                                xt[h][k][:, mc * 128 : (mc + 1) * 128],
                                wsb[5][k][:, nh * 512 : (nh + 1) * 512],
                                start=(k == 0),
                                stop=(k == NK - 1),
                            )
                        tmp = tpool.tile([128, 512], f32, tag="tmp")
                        nc.vector.tensor_add(
                            tmp[:], ps[:], bias_sb[:, nh * 512 : (nh + 1) * 512]
                        )
                        o = opool.tile([128, 512], f32, tag="o")
                        nc.scalar.activation(o[:], tmp[:], ActivationFunctionType.Relu)
                        b0 = h * HB + mc * 4
                        eng = nc.sync if (mc + nh) % 2 == 0 else nc.scalar
                        eng.dma_start(
                            out=y3[b0 : b0 + 4, :, nh * 512 : (nh + 1) * 512],
                            in_=o[:],
                        )
    # Run the Bacc lowering passes (register allocation, wait splitting, ...)
    # — the PJRT execute path serializes nc.m as-is.
    nc.finalize()
    return nc


def _get_nc(mm_dtype_name):
    if mm_dtype_name not in _NC_CACHE:
        _NC_CACHE[mm_dtype_name] = _build_nc(mm_dtype_name)
    return _NC_CACHE[mm_dtype_name]


def _prep_weights(nodes, npdt):
    # W[i] layout [p=(u*32+v)%... rows 128 per k-tile packed as [128, 8*1024]]:
    # free index = k*1024 + col.  steps 0..4: col = n*32+m ; step 5: col = m*32+n.
    nodes = np.ascontiguousarray(nodes, dtype=np.float32)
    W = np.empty((NSTEP, 128, 8192), np.float32)
    for i in range(NSTEP):
        if i < 5:
            wm = nodes[i].reshape(1024, 32, 32).transpose(0, 2, 1).reshape(1024, 1024)
        else:
            wm = nodes[i].reshape(1024, 1024)
        # [k*128+p, col] -> [p, k*1024+col]
        W[i] = wm.reshape(NK, 128, 1024).transpose(1, 0, 2).reshape(128, 8192)
    return np.ascontiguousarray(W.astype(npdt))


def _prep_x(xc, npdt):
    # [BC, 32768] -> [k, p, b*32+f] with value x[b, (k*128+p)*32+f]
    xp = xc.reshape(BC, NK, 128, 32).transpose(1, 2, 0, 3).reshape(NK, 128, BC * 32)
    return np.ascontiguousarray(xp.astype(npdt))


def run(inputs, nodes, bias, mm_dtype="bfloat16", trace=False):
    from concourse.bass_utils import run_bass_kernel_spmd

    nc = _get_nc(mm_dtype)
    npdt = _np_dtype(mm_dtype)
    x = np.ascontiguousarray(inputs, dtype=np.float32)
    bias = np.ascontiguousarray(bias, dtype=np.float32)
    W = _prep_weights(nodes, npdt)
    in_maps = [
        {
            "x": _prep_x(x[c * BC : (c + 1) * BC], npdt),
            "w": W,
            "bias_in": bias,
        }
        for c in range(NCORES)
    ]
    res = run_bass_kernel_spmd(nc, in_maps, list(range(NCORES)), trace=trace)
    out = np.concatenate([res.results[c]["y"] for c in range(NCORES)], axis=0)
    return out, res


def kernel(inputs, nodes, bias):
    out, _ = run(inputs, nodes, bias)
    return out
